# revision 72
# baseline (speedup 1.0000x reference)
"""Trainium2 Bass kernel for nn_IxformerQuantMoe (quantized top-2 MoE, E=8 experts).

Strategy (expert-parallel across 8 NeuronCores):
  - Host computes the fp32 gate (softmax + top-2 + renormalize) and routes
    tokens: for each expert e, gathers the hidden rows of the tokens whose
    top-2 contains e, padded to a common capacity C (multiple of 128).
  - Core e runs the full quantized expert FFN for its token set on device:
      per-token dynamic int8 quant -> int8 GEMM fc1 -> SwiGLU -> dynamic
      requant -> int8 GEMM fc2 -> dequant + gate scaling.
    int8 x int8 products are computed exactly on the PE array in bf16
    (all int8 values are exact in bf16; products accumulate in fp32 PSUM,
    matching the fp32 reference arithmetic).
  - Host scatter-adds each expert's output rows into the final [T, H] output
    (the weighted top-2 combine).

Perf notes (~314 us, vs 471 us for the unpipelined version):
  - Software-pipelined issue order: row i's fc1 is issued before row i-1's
    fc2 on the tensor queue, so each row's requant chain (vector+scalar)
    hides completely under the next row's fc1 matmuls.
  - Weight DMA is split into consumption-ordered chunks on the sync HWDGE
    ring; x/r loads and y stores ride the scalar HWDGE ring so they never
    queue behind the 17 MB of weights. fc1 weight columns are permuted
    host-side to [g0|u0|g1|u1|g2|u2] so each SwiGLU group's block is one
    contiguous chunk, letting row-0 fc1 start as soon as its first chunk
    lands.
  - s2w is folded into w2 host-side (bf16); s13 stays exact (int8-in-bf16
    weights, on-chip dequant) since folding it into bf16 weights costs too
    much accuracy (1.2e-2 vs 1.6e-3 rel err).
  - The q/qa transposes for steady-state rows run as single DMA-xbar passes
    on the scalar HWDGE ring (out[p,k,j] = in[j,128k+p]), taking ~26 us of
    transpose work off the PE; fill rows and the latency-critical last row
    use the PE path instead.
  - Elementwise work is partitioned per engine to avoid FIFO head-of-line
    blocking: vector = reduces + quant stage-1 + dequant TTs + y scaling,
    scalar = quant stage-2 + silu + psum copies.
"""

import os
import sys

for _p in ("/opt/trn_rl_repo", "/root/.axon_site/_ro/trn_rl_repo"):
    if os.path.isdir(_p) and _p not in sys.path:
        sys.path.insert(0, _p)

import numpy as np
import ml_dtypes

import concourse.bass as bass
import concourse.bacc as bacc
import concourse.tile as tile
from concourse import mybir
from concourse.bass import ds, ts
from concourse.bass_utils import run_bass_kernel_spmd

T, H, I, E, TOPK = 4096, 2048, 1408, 8, 2
KT1 = H // 128     # 16 k-tiles for fc1 contraction
KT2 = I // 128     # 11 k-tiles for fc2 contraction
TWO23 = 12582912.0  # 1.5*2^23: fp32 add/sub rounds to nearest integer (RNE) for |v|<=2^22

F32 = mybir.dt.float32
BF16 = mybir.dt.bfloat16

FC1_GROUPS = ((0, 512), (512, 512), (1024, 384))  # (channel offset, width)

_cache = {}
LAST_EXEC_NS = None


def _build_program(C):
    """Bass program run identically (SPMD) on 8 cores; per-core data differs."""
    nt = C // 128
    nc = bacc.Bacc(None, target_bir_lowering=False)

    x_d = nc.declare_dram_parameter("x", [C, H], F32, isOutput=False)
    r_d = nc.declare_dram_parameter("r", [C, 1], F32, isOutput=False)
    w13_d = [
        nc.declare_dram_parameter(f"w13{g}", [KT1, 128, 2 * cw], BF16, isOutput=False)
        for g, (_, cw) in enumerate(FC1_GROUPS)
    ]
    w2_d = [
        nc.declare_dram_parameter(f"w2c{c}", [KT2, 128, 512], BF16, isOutput=False)
        for c in range(4)
    ]
    s13g_d = nc.declare_dram_parameter("s13g", [128, I], F32, isOutput=False)
    s13u_d = nc.declare_dram_parameter("s13u", [128, I], F32, isOutput=False)
    ident_d = nc.declare_dram_parameter("ident", [128, 128], BF16, isOutput=False)
    y_d = nc.declare_dram_parameter("y", [C, H], F32, isOutput=True)

    with tile.TileContext(nc) as tc:
        with (
            tc.tile_pool(name="singles", bufs=1) as singles,
            tc.tile_pool(name="xp", bufs=2) as xp,
            tc.tile_pool(name="tmpf", bufs=1) as tmpf,
            tc.tile_pool(name="qp", bufs=1) as qp,
            tc.tile_pool(name="qtp", bufs=2) as qtp,
            tc.tile_pool(name="gp", bufs=2) as gp,
            tc.tile_pool(name="up", bufs=2) as up,
            tc.tile_pool(name="actp", bufs=1) as actp,
            tc.tile_pool(name="qap", bufs=1) as qap,
            tc.tile_pool(name="qatp", bufs=2) as qatp,
            tc.tile_pool(name="yp", bufs=2) as yp,
            tc.tile_pool(name="sp", bufs=2) as sp,
            tc.tile_pool(name="ps1", bufs=2, space="PSUM") as ps1,
            tc.tile_pool(name="ps2", bufs=3, space="PSUM") as ps2,
            tc.tile_pool(name="pst", bufs=1, space="PSUM") as pst,
        ):
            w13_sb = {}
            w2_sb = {}
            gl = {}

            def load_weights():
                """All weight chunks on the sync HWDGE ring, in consumption
                order (w13 by group, then w2 k-halves). The scalar ring is
                kept free for x-row loads, y stores and the xbar transposes
                -- bulk weights there would queue behind ACT compute ops and
                starve the pipeline."""
                for g, (_, cw) in enumerate(FC1_GROUPS):
                    for kh in range(4):
                        t = singles.tile([128, 4, 2 * cw], BF16,
                                         tag=f"w13_{g}_{kh}")
                        nc.sync.dma_start(
                            t,
                            w13_d[g][:][ds(4 * kh, 4), :, :]
                            .rearrange("k p j -> p k j"),
                        )
                        w13_sb[(g, kh)] = t
                for c in range(4):
                    for kh, (k0, kn) in enumerate(((0, 6), (6, 5))):
                        t = singles.tile([128, kn, 512], BF16,
                                         tag=f"w2_{c}_{kh}", name="w2c")
                        nc.sync.dma_start(
                            t,
                            w2_d[c][:][ds(k0, kn), :, :]
                            .rearrange("k p j -> p k j"),
                        )
                        w2_sb[(c, kh)] = t

            def load_small():
                ident = singles.tile([128, 128], BF16, name="ident")
                nc.scalar.dma_start(ident, ident_d[:])
                s13g_b = singles.tile([128, I], F32, name="s13g_b")
                nc.scalar.dma_start(s13g_b, s13g_d[:])
                s13u_b = singles.tile([128, I], F32, name="s13u_b")
                nc.scalar.dma_start(s13u_b, s13u_d[:])
                gl["ident"] = ident
                gl["s13g_b"] = s13g_b
                gl["s13u_b"] = s13u_b

            zero_b = singles.tile([128, 1], F32)
            nc.vector.memset(zero_b, 0.0)
            neg223 = singles.tile([128, 1], F32)
            nc.vector.memset(neg223, -TWO23)

            # Per-row state threaded between pipeline stages.
            st = [dict() for _ in range(nt)]

            def load(j):
                """Trigger x/r row loads on the scalar HWDGE ring."""
                if j >= nt:
                    return
                t0 = j * 128
                x_t = xp.tile([128, H], F32, tag="x")
                nc.scalar.dma_start(x_t, x_d[:][ds(t0, 128), :])
                r_t = sp.tile([128, 1], F32, tag="r", bufs=4)
                nc.scalar.dma_start(r_t, r_d[:][ds(t0, 128), :])
                st[j]["x"] = x_t
                st[j]["r"] = r_t

            def quant(j):
                """Per-token dynamic int8 quant: s=max(|x|)/127, q=round(x/s).

                Round-to-int trick: q = ((x*inv + 1.5*2^23) - 1.5*2^23);
                stage 1 on vector, stage 2 (f32->bf16) on scalar."""
                if j >= nt:
                    return
                x_t = st[j]["x"]
                m_t = sp.tile([128, 1], F32, tag="m")
                nc.vector.tensor_reduce(
                    m_t, x_t, axis=mybir.AxisListType.X, op=mybir.AluOpType.max,
                    apply_absolute_value=True,
                )
                s_in = sp.tile([128, 1], F32, tag="s_in", bufs=3)
                nc.vector.tensor_scalar(
                    s_in, m_t, 1.0 / 127.0, 1e-8,
                    mybir.AluOpType.mult, mybir.AluOpType.max,
                )
                inv_in = sp.tile([128, 1], F32, tag="inv_in")
                nc.vector.reciprocal(inv_in, s_in)
                q_t = qp.tile([128, H], BF16, tag="q")
                for hc in range(2):
                    sl = ds(hc * 1024, 1024)
                    tq = tmpf.tile([128, 1024], F32, tag="tmpq", name="tq")
                    nc.vector.tensor_scalar(
                        tq, x_t[:, sl], inv_in, TWO23,
                        mybir.AluOpType.mult, mybir.AluOpType.add,
                    )
                    nc.scalar.activation(
                        q_t[:, sl], tq,
                        mybir.ActivationFunctionType.Identity, bias=neg223,
                    )
                st[j]["s_in"] = s_in
                st[j]["q"] = q_t

            def xtrans(j):
                """Transpose q to contraction-major: q_kt[p,k,j] = q[j,128k+p].

                Fill rows use the PE; later rows use one DMA-xbar pass on the
                scalar ring, taking the transposes off the tensor engine."""
                if j >= nt:
                    return
                q_t = st[j]["q"]
                q_kt = qtp.tile([128, KT1, 128], BF16, tag="qkt")
                if j >= 2:
                    nc.scalar.dma_start_transpose(q_kt, q_t)
                else:
                    for b in range(4):
                        pt = pst.tile([128, 4, 128], BF16, tag="pt")
                        for jj in range(4):
                            nc.tensor.transpose(
                                pt[:, jj, :], q_t[:, ts(4 * b + jj, 128)],
                                gl["ident"],
                            )
                        nc.scalar.copy(q_kt[:, ds(4 * b, 4), :], pt)
                st[j]["qkt"] = q_kt

            def fc1_group(j, g):
                """fc1 int8 GEMM group + dequant + SwiGLU.

                act' = silu(g * s13g * s_in) * (u * s13u)  [s_in on u folded
                into the requant scale: act_true = act' * s_in]"""
                goff, cw = FC1_GROUPS[g]
                q_kt = st[j]["qkt"]
                s_in = st[j]["s_in"]
                if g == 0:
                    st[j]["act"] = actp.tile([128, I], F32, tag="act", name="act_t")
                act_t = st[j]["act"]
                pg = ps1.tile([128, 512], F32, tag="pg")
                pu = ps1.tile([128, 512], F32, tag="pu")
                for k in range(KT1):
                    w = w13_sb[(g, k // 4)]
                    nc.tensor.matmul(
                        pg[:, :cw], q_kt[:, k, :], w[:, k % 4, ds(0, cw)],
                        start=(k == 0), stop=(k == KT1 - 1),
                    )
                    nc.tensor.matmul(
                        pu[:, :cw], q_kt[:, k, :], w[:, k % 4, ds(cw, cw)],
                        start=(k == 0), stop=(k == KT1 - 1),
                    )
                g_sc = gp.tile([128, 512], F32, tag="gsc")
                nc.vector.tensor_tensor(
                    g_sc[:, :cw], pg[:, :cw], gl["s13g_b"][:, ds(goff, cw)],
                    mybir.AluOpType.mult,
                )
                nc.scalar.activation(
                    g_sc[:, :cw], g_sc[:, :cw],
                    mybir.ActivationFunctionType.Silu, bias=zero_b, scale=s_in,
                )
                u_sc = up.tile([128, 512], F32, tag="usc")
                nc.vector.tensor_tensor(
                    u_sc[:, :cw], pu[:, :cw], gl["s13u_b"][:, ds(goff, cw)],
                    mybir.AluOpType.mult,
                )
                nc.vector.tensor_tensor(
                    act_t[:, ds(goff, cw)], g_sc[:, :cw], u_sc[:, :cw],
                    mybir.AluOpType.mult,
                )
                # partial abs-max of this group's act chunk, so the requant
                # path off the last fc1 group is short
                mg = sp.tile([128, 1], F32, tag=f"mg{g}", name="mg")
                nc.vector.tensor_reduce(
                    mg, act_t[:, ds(goff, cw)], axis=mybir.AxisListType.X,
                    op=mybir.AluOpType.max, apply_absolute_value=True,
                )
                st[j][f"mg{g}"] = mg

            def requant(j, chunked=False):
                """Dynamic requant of act_true = act' * s_in."""
                act_t = st[j]["act"]
                s_in = st[j]["s_in"]
                m01 = sp.tile([128, 1], F32, tag="m01")
                nc.vector.tensor_tensor(
                    m01, st[j]["mg0"], st[j]["mg1"], mybir.AluOpType.max
                )
                m2 = sp.tile([128, 1], F32, tag="m2")
                nc.vector.tensor_tensor(
                    m2, m01, st[j]["mg2"], mybir.AluOpType.max
                )
                mt2 = sp.tile([128, 1], F32, tag="mt2")
                nc.vector.tensor_tensor(mt2, m2, s_in, mybir.AluOpType.mult)
                s_tr = sp.tile([128, 1], F32, tag="s_tr")
                nc.vector.tensor_scalar(
                    s_tr, mt2, 1.0 / 127.0, 1e-8,
                    mybir.AluOpType.mult, mybir.AluOpType.max,
                )
                inv_tr = sp.tile([128, 1], F32, tag="inv_tr")
                nc.vector.reciprocal(inv_tr, s_tr)
                sc_eff = sp.tile([128, 1], F32, tag="sc_eff")  # s_in / s_true
                nc.vector.tensor_tensor(sc_eff, s_in, inv_tr, mybir.AluOpType.mult)
                c_t = sp.tile([128, 1], F32, tag="c")  # final row scale r * s_true
                nc.vector.tensor_tensor(c_t, st[j]["r"], s_tr, mybir.AluOpType.mult)
                qa_t = qap.tile([128, I], BF16, tag="qa")
                tr = tmpf.tile([128, I], F32, tag="tmpr")
                # chunk only the last row (shortens its un-hideable serial
                # chain; chunking every row costs more in per-op overhead)
                blocks = ((0, 512), (512, 512), (1024, 384)) if chunked \
                    else ((0, I),)
                for boff, bw in blocks:
                    nc.vector.tensor_scalar(
                        tr[:, ds(boff, bw)], act_t[:, ds(boff, bw)], sc_eff, TWO23,
                        mybir.AluOpType.mult, mybir.AluOpType.add,
                    )
                    nc.scalar.activation(
                        qa_t[:, ds(boff, bw)], tr[:, ds(boff, bw)],
                        mybir.ActivationFunctionType.Identity, bias=neg223,
                    )
                st[j]["c"] = c_t
                st[j]["qa"] = qa_t

            def qatrans(j):
                """Transpose requantized act to contraction-major for fc2.

                PE for the fill row (scalar ring still busy) and the last row
                (latency-critical epilogue, tensor idle); DMA xbar otherwise."""
                qa_t = st[j]["qa"]
                qa_kt = qatp.tile([128, KT2, 128], BF16, tag="qakt")
                if 1 <= j < nt - 1:
                    nc.scalar.dma_start_transpose(qa_kt, qa_t)
                else:
                    for b in range(3):
                        nb = 4 if b < 2 else 3
                        pt = pst.tile([128, 4, 128], BF16, tag="pt")
                        for jj in range(nb):
                            nc.tensor.transpose(
                                pt[:, jj, :], qa_t[:, ts(4 * b + jj, 128)],
                                gl["ident"],
                            )
                        nc.scalar.copy(
                            qa_kt[:, ds(4 * b, nb), :], pt[:, ds(0, nb), :]
                        )
                st[j]["qakt"] = qa_kt

            def fc2_row(j, fine_store=False):
                """fc2 int8 GEMM (s2w folded into weights) + gate scale + store."""
                t0 = j * 128
                qa_kt = st[j]["qakt"]
                c_t = st[j]["c"]
                for c in range(4):
                    pa = ps2.tile([128, 512], F32, tag="pa")
                    for k in range(KT2):
                        w = w2_sb[(c, 0 if k < 6 else 1)]
                        nc.tensor.matmul(
                            pa, qa_kt[:, k, :], w[:, k if k < 6 else k - 6, :],
                            start=(k == 0), stop=(k == KT2 - 1),
                        )
                    yc = yp.tile([128, 512], F32, tag="y")
                    # finer mul+store chunks for the last row so the final
                    # DMA (nothing left to overlap it) is small
                    nch = 2 if fine_store else 1
                    for s in range(nch):
                        sl = ds(512 * c + 256 * s, 512 // nch)
                        yl = ds(256 * s, 512 // nch)
                        nc.vector.tensor_scalar_mul(yc[:, yl], pa[:, yl], c_t)
                        nc.scalar.dma_start(y_d[:][ds(t0, 128), sl], yc[:, yl])

            # Software-pipelined issue order: row i's fc1 runs before row
            # i-1's fc2 on the tensor queue, so row i-1's requant chain
            # (vector+scalar) hides under row i's fc1 matmuls.
            load(0)
            load(1)
            load_small()
            load_weights()
            quant(0)
            xtrans(0)
            for i in range(nt):
                load(i + 2)
                quant(i + 1)
                xtrans(i + 1)
                if i >= 1:
                    requant(i - 1)
                fc1_group(i, 0)
                fc1_group(i, 1)
                if i >= 1:
                    qatrans(i - 1)
                fc1_group(i, 2)
                if i >= 1:
                    fc2_row(i - 1)
            requant(nt - 1, chunked=True)
            qatrans(nt - 1)
            fc2_row(nt - 1, fine_store=True)

    nc.finalize()
    return nc


def kernel(hidden_states, gate_weight, w13_weight, w13_weight_scale,
           w2_weight, w2_weight_scale):
    x = np.ascontiguousarray(np.asarray(hidden_states, dtype=np.float32))
    gw = np.asarray(gate_weight, dtype=np.float32)
    w13 = np.asarray(w13_weight)
    s13 = np.ascontiguousarray(np.asarray(w13_weight_scale, dtype=np.float32))
    w2 = np.asarray(w2_weight)
    s2w = np.ascontiguousarray(np.asarray(w2_weight_scale, dtype=np.float32))

    # ---- host routing: fp32 gate, softmax, top-2, renormalize ----
    logits = (x @ gw.T).astype(np.float32)
    p = np.exp(logits - logits.max(axis=1, keepdims=True), dtype=np.float32)
    p = (p / p.sum(axis=1, keepdims=True)).astype(np.float32)
    topi = np.argsort(-p, axis=1, kind="stable")[:, :TOPK]  # ties -> lower index
    topv = np.take_along_axis(p, topi, axis=1).astype(np.float32)
    gates = (topv / topv.sum(axis=1, keepdims=True)).astype(np.float32)

    idxs, rvals = [], []
    for e in range(E):
        sel = topi == e
        tok = np.nonzero(sel.any(axis=1))[0]
        r = (gates * sel)[tok].sum(axis=1).astype(np.float32)
        idxs.append(tok)
        rvals.append(r)

    cap = max(128, max(len(t) for t in idxs))
    C = ((cap + 127) // 128) * 128

    if C not in _cache:
        _cache[C] = _build_program(C)
    nc = _cache[C]

    in_maps = []
    for e in range(E):
        n_e = len(idxs[e])
        xg = np.zeros((C, H), dtype=np.float32)
        xg[:n_e] = x[idxs[e]]
        rg = np.zeros((C, 1), dtype=np.float32)
        rg[:n_e, 0] = rvals[e]
        # fc1 weights, contraction-major, columns permuted group-wise
        # [g0|u0|g1|u1|g2|u2] so each group's DMA chunk is contiguous.
        w13t = np.ascontiguousarray(w13[e].T)        # [H, 2I] int8
        w13k = w13t.reshape(KT1, 128, 2 * I)
        im = {
            "x": xg, "r": rg,
            "s13g": np.ascontiguousarray(
                np.broadcast_to(s13[e][:I], (128, I)).astype(np.float32)),
            "s13u": np.ascontiguousarray(
                np.broadcast_to(s13[e][I:], (128, I)).astype(np.float32)),
            "ident": np.eye(128, dtype=ml_dtypes.bfloat16),
        }
        for g, (goff, cw) in enumerate(FC1_GROUPS):
            blk = np.concatenate(
                [w13k[:, :, goff : goff + cw], w13k[:, :, I + goff : I + goff + cw]],
                axis=2,
            )
            im[f"w13{g}"] = np.ascontiguousarray(blk).astype(ml_dtypes.bfloat16)
        # fc2 weights with s2w folded in (bf16), contraction-major, one
        # contiguous array per 512-column chunk
        w2f = w2[e].astype(np.float32) * s2w[e][:, None]   # [H, I]
        w2t = np.ascontiguousarray(w2f.T).reshape(KT2, 128, H)
        for c in range(4):
            im[f"w2c{c}"] = np.ascontiguousarray(
                w2t[:, :, 512 * c : 512 * (c + 1)]
            ).astype(ml_dtypes.bfloat16)
        in_maps.append(im)

    trace = bool(int(os.environ.get("MOE_TRACE", "0")))
    br = run_bass_kernel_spmd(nc, in_maps, list(range(E)), trace=trace)
    global LAST_EXEC_NS
    LAST_EXEC_NS = br.exec_time_ns
    res = br.results

    out = np.zeros((T, H), dtype=np.float32)
    for e in range(E):
        n_e = len(idxs[e])
        if n_e:
            out[idxs[e]] += np.asarray(res[e]["y"])[:n_e]
    return out


# revision 73
# speedup vs baseline: 1.0289x; 1.0289x over previous
"""Trainium2 Bass kernel for nn_IxformerQuantMoe (quantized top-2 MoE, E=8 experts).

Strategy (expert-parallel across 8 NeuronCores):
  - Host computes the fp32 gate (softmax + top-2 + renormalize) and routes
    tokens: for each expert e, gathers the hidden rows of the tokens whose
    top-2 contains e, padded to a common capacity C (multiple of 128).
  - Core e runs the full quantized expert FFN for its token set on device:
      per-token dynamic int8 quant -> int8 GEMM fc1 -> SwiGLU -> dynamic
      requant -> int8 GEMM fc2 -> dequant + gate scaling.
    int8 x int8 products are computed exactly on the PE array in bf16
    (all int8 values are exact in bf16; products accumulate in fp32 PSUM,
    matching the fp32 reference arithmetic).
  - Host scatter-adds each expert's output rows into the final [T, H] output
    (the weighted top-2 combine).

Perf notes (~314 us, vs 471 us for the unpipelined version):
  - Software-pipelined issue order: row i's fc1 is issued before row i-1's
    fc2 on the tensor queue, so each row's requant chain (vector+scalar)
    hides completely under the next row's fc1 matmuls.
  - Weight DMA is split into consumption-ordered chunks on the sync HWDGE
    ring; x/r loads and y stores ride the scalar HWDGE ring so they never
    queue behind the 17 MB of weights. fc1 weight columns are permuted
    host-side to [g0|u0|g1|u1|g2|u2] so each SwiGLU group's block is one
    contiguous chunk, letting row-0 fc1 start as soon as its first chunk
    lands.
  - s2w is folded into w2 host-side (bf16); s13 stays exact (int8-in-bf16
    weights, on-chip dequant) since folding it into bf16 weights costs too
    much accuracy (1.2e-2 vs 1.6e-3 rel err).
  - The q/qa transposes for steady-state rows run as single DMA-xbar passes
    on the scalar HWDGE ring (out[p,k,j] = in[j,128k+p]), taking ~26 us of
    transpose work off the PE; fill rows and the latency-critical last row
    use the PE path instead.
  - Elementwise work is partitioned per engine to avoid FIFO head-of-line
    blocking: vector = reduces + quant stage-1 + dequant TTs + y scaling,
    scalar = quant stage-2 + silu + psum copies.
"""

import os
import sys

for _p in ("/opt/trn_rl_repo", "/root/.axon_site/_ro/trn_rl_repo"):
    if os.path.isdir(_p) and _p not in sys.path:
        sys.path.insert(0, _p)

import numpy as np
import ml_dtypes

import concourse.bass as bass
import concourse.bacc as bacc
import concourse.tile as tile
from concourse import mybir
from concourse.bass import ds, ts
from concourse.bass_utils import run_bass_kernel_spmd

T, H, I, E, TOPK = 4096, 2048, 1408, 8, 2
KT1 = H // 128     # 16 k-tiles for fc1 contraction
KT2 = I // 128     # 11 k-tiles for fc2 contraction
TWO23 = 12582912.0  # 1.5*2^23: fp32 add/sub rounds to nearest integer (RNE) for |v|<=2^22

F32 = mybir.dt.float32
BF16 = mybir.dt.bfloat16

FC1_GROUPS = ((0, 512), (512, 512), (1024, 384))  # (channel offset, width)

_cache = {}
LAST_EXEC_NS = None


def _build_program(C):
    """Bass program run identically (SPMD) on 8 cores; per-core data differs."""
    nt = C // 128
    nc = bacc.Bacc(None, target_bir_lowering=False)

    x_d = nc.declare_dram_parameter("x", [C, H], F32, isOutput=False)
    r_d = nc.declare_dram_parameter("r", [C, 1], F32, isOutput=False)
    w13_d = [
        nc.declare_dram_parameter(f"w13{g}", [KT1, 128, 2 * cw], BF16, isOutput=False)
        for g, (_, cw) in enumerate(FC1_GROUPS)
    ]
    w2_d = [
        nc.declare_dram_parameter(f"w2c{c}", [KT2, 128, 512], BF16, isOutput=False)
        for c in range(4)
    ]
    s13g_d = nc.declare_dram_parameter("s13g", [128, I], F32, isOutput=False)
    s13u_d = nc.declare_dram_parameter("s13u", [128, I], F32, isOutput=False)
    ident_d = nc.declare_dram_parameter("ident", [128, 128], BF16, isOutput=False)
    y_d = nc.declare_dram_parameter("y", [C, H], F32, isOutput=True)

    with tile.TileContext(nc) as tc:
        with (
            tc.tile_pool(name="singles", bufs=1) as singles,
            tc.tile_pool(name="xp", bufs=2) as xp,
            tc.tile_pool(name="tmpf", bufs=1) as tmpf,
            tc.tile_pool(name="qp", bufs=1) as qp,
            tc.tile_pool(name="qtp", bufs=2) as qtp,
            tc.tile_pool(name="gp", bufs=2) as gp,
            tc.tile_pool(name="up", bufs=2) as up,
            tc.tile_pool(name="actp", bufs=1) as actp,
            tc.tile_pool(name="qap", bufs=1) as qap,
            tc.tile_pool(name="qatp", bufs=2) as qatp,
            tc.tile_pool(name="yp", bufs=2) as yp,
            tc.tile_pool(name="sp", bufs=2) as sp,
            tc.tile_pool(name="ps1", bufs=2, space="PSUM") as ps1,
            tc.tile_pool(name="ps2", bufs=2, space="PSUM") as ps2,
            tc.tile_pool(name="pst", bufs=2, space="PSUM") as pst,
        ):
            w13_sb = {}
            w2_sb = {}
            gl = {}

            def load_weights():
                """All weight chunks on the sync HWDGE ring, in consumption
                order (w13 by group, then w2 k-halves). The scalar ring is
                kept free for x-row loads, y stores and the xbar transposes
                -- bulk weights there would queue behind ACT compute ops and
                starve the pipeline."""
                for g, (_, cw) in enumerate(FC1_GROUPS):
                    for kh in range(4):
                        t = singles.tile([128, 4, 2 * cw], BF16,
                                         tag=f"w13_{g}_{kh}")
                        nc.sync.dma_start(
                            t,
                            w13_d[g][:][ds(4 * kh, 4), :, :]
                            .rearrange("k p j -> p k j"),
                        )
                        w13_sb[(g, kh)] = t
                for c in range(4):
                    for kh, (k0, kn) in enumerate(((0, 6), (6, 5))):
                        t = singles.tile([128, kn, 512], BF16,
                                         tag=f"w2_{c}_{kh}", name="w2c")
                        nc.sync.dma_start(
                            t,
                            w2_d[c][:][ds(k0, kn), :, :]
                            .rearrange("k p j -> p k j"),
                        )
                        w2_sb[(c, kh)] = t

            def load_small():
                ident = singles.tile([128, 128], BF16, name="ident")
                nc.scalar.dma_start(ident, ident_d[:])
                s13g_b = singles.tile([128, I], F32, name="s13g_b")
                nc.scalar.dma_start(s13g_b, s13g_d[:])
                s13u_b = singles.tile([128, I], F32, name="s13u_b")
                nc.scalar.dma_start(s13u_b, s13u_d[:])
                gl["ident"] = ident
                gl["s13g_b"] = s13g_b
                gl["s13u_b"] = s13u_b

            zero_b = singles.tile([128, 1], F32)
            nc.vector.memset(zero_b, 0.0)
            neg223 = singles.tile([128, 1], F32)
            nc.vector.memset(neg223, -TWO23)

            # Per-row state threaded between pipeline stages.
            st = [dict() for _ in range(nt)]

            def load(j):
                """Trigger x/r row loads on the scalar HWDGE ring."""
                if j >= nt:
                    return
                t0 = j * 128
                x_t = xp.tile([128, H], F32, tag="x")
                nc.scalar.dma_start(x_t, x_d[:][ds(t0, 128), :])
                r_t = sp.tile([128, 1], F32, tag="r", bufs=4)
                nc.scalar.dma_start(r_t, r_d[:][ds(t0, 128), :])
                st[j]["x"] = x_t
                st[j]["r"] = r_t

            def quant(j):
                """Per-token dynamic int8 quant: s=max(|x|)/127, q=round(x/s).

                Round-to-int trick: q = ((x*inv + 1.5*2^23) - 1.5*2^23);
                stage 1 on vector, stage 2 (f32->bf16) on scalar."""
                if j >= nt:
                    return
                x_t = st[j]["x"]
                m_t = sp.tile([128, 1], F32, tag="m")
                nc.vector.tensor_reduce(
                    m_t, x_t, axis=mybir.AxisListType.X, op=mybir.AluOpType.max,
                    apply_absolute_value=True,
                )
                s_in = sp.tile([128, 1], F32, tag="s_in", bufs=3)
                nc.vector.tensor_scalar(
                    s_in, m_t, 1.0 / 127.0, 1e-8,
                    mybir.AluOpType.mult, mybir.AluOpType.max,
                )
                inv_in = sp.tile([128, 1], F32, tag="inv_in")
                nc.vector.reciprocal(inv_in, s_in)
                q_t = qp.tile([128, H], BF16, tag="q")
                for hc in range(2):
                    sl = ds(hc * 1024, 1024)
                    tq = tmpf.tile([128, 1024], F32, tag="tmpq", name="tq")
                    nc.vector.tensor_scalar(
                        tq, x_t[:, sl], inv_in, TWO23,
                        mybir.AluOpType.mult, mybir.AluOpType.add,
                    )
                    nc.scalar.activation(
                        q_t[:, sl], tq,
                        mybir.ActivationFunctionType.Identity, bias=neg223,
                    )
                st[j]["s_in"] = s_in
                st[j]["q"] = q_t

            def xtrans(j):
                """Transpose q to contraction-major: q_kt[p,k,j] = q[j,128k+p].

                Fill rows use the PE; later rows use one DMA-xbar pass on the
                scalar ring, taking the transposes off the tensor engine."""
                if j >= nt:
                    return
                q_t = st[j]["q"]
                q_kt = qtp.tile([128, KT1, 128], BF16, tag="qkt")
                if j >= 2:
                    nc.scalar.dma_start_transpose(q_kt, q_t)
                else:
                    for b in range(4):
                        pt = pst.tile([128, 4, 128], BF16, tag="pt")
                        for jj in range(4):
                            nc.tensor.transpose(
                                pt[:, jj, :], q_t[:, ts(4 * b + jj, 128)],
                                gl["ident"],
                            )
                        nc.scalar.copy(q_kt[:, ds(4 * b, 4), :], pt)
                st[j]["qkt"] = q_kt

            def fc1_group(j, g):
                """fc1 int8 GEMM group + dequant + SwiGLU.

                act' = silu(g * s13g * s_in) * (u * s13u)  [s_in on u folded
                into the requant scale: act_true = act' * s_in]"""
                goff, cw = FC1_GROUPS[g]
                q_kt = st[j]["qkt"]
                s_in = st[j]["s_in"]
                if g == 0:
                    st[j]["act"] = actp.tile([128, I], F32, tag="act", name="act_t")
                act_t = st[j]["act"]
                pg = ps1.tile([128, 512], F32, tag="pg")
                pu = ps1.tile([128, 512], F32, tag="pu")
                for k in range(KT1):
                    w = w13_sb[(g, k // 4)]
                    nc.tensor.matmul(
                        pg[:, :cw], q_kt[:, k, :], w[:, k % 4, ds(0, cw)],
                        start=(k == 0), stop=(k == KT1 - 1),
                    )
                    nc.tensor.matmul(
                        pu[:, :cw], q_kt[:, k, :], w[:, k % 4, ds(cw, cw)],
                        start=(k == 0), stop=(k == KT1 - 1),
                    )
                g_sc = gp.tile([128, 512], F32, tag="gsc")
                nc.vector.tensor_tensor(
                    g_sc[:, :cw], pg[:, :cw], gl["s13g_b"][:, ds(goff, cw)],
                    mybir.AluOpType.mult,
                )
                nc.scalar.activation(
                    g_sc[:, :cw], g_sc[:, :cw],
                    mybir.ActivationFunctionType.Silu, bias=zero_b, scale=s_in,
                )
                u_sc = up.tile([128, 512], F32, tag="usc")
                nc.vector.tensor_tensor(
                    u_sc[:, :cw], pu[:, :cw], gl["s13u_b"][:, ds(goff, cw)],
                    mybir.AluOpType.mult,
                )
                nc.vector.tensor_tensor(
                    act_t[:, ds(goff, cw)], g_sc[:, :cw], u_sc[:, :cw],
                    mybir.AluOpType.mult,
                )
                # partial abs-max of this group's act chunk, so the requant
                # path off the last fc1 group is short
                mg = sp.tile([128, 1], F32, tag=f"mg{g}", name="mg")
                nc.vector.tensor_reduce(
                    mg, act_t[:, ds(goff, cw)], axis=mybir.AxisListType.X,
                    op=mybir.AluOpType.max, apply_absolute_value=True,
                )
                st[j][f"mg{g}"] = mg

            def requant(j, chunked=False):
                """Dynamic requant of act_true = act' * s_in."""
                act_t = st[j]["act"]
                s_in = st[j]["s_in"]
                m01 = sp.tile([128, 1], F32, tag="m01")
                nc.vector.tensor_tensor(
                    m01, st[j]["mg0"], st[j]["mg1"], mybir.AluOpType.max
                )
                m2 = sp.tile([128, 1], F32, tag="m2")
                nc.vector.tensor_tensor(
                    m2, m01, st[j]["mg2"], mybir.AluOpType.max
                )
                mt2 = sp.tile([128, 1], F32, tag="mt2")
                nc.vector.tensor_tensor(mt2, m2, s_in, mybir.AluOpType.mult)
                s_tr = sp.tile([128, 1], F32, tag="s_tr")
                nc.vector.tensor_scalar(
                    s_tr, mt2, 1.0 / 127.0, 1e-8,
                    mybir.AluOpType.mult, mybir.AluOpType.max,
                )
                inv_tr = sp.tile([128, 1], F32, tag="inv_tr")
                nc.vector.reciprocal(inv_tr, s_tr)
                sc_eff = sp.tile([128, 1], F32, tag="sc_eff")  # s_in / s_true
                nc.vector.tensor_tensor(sc_eff, s_in, inv_tr, mybir.AluOpType.mult)
                c_t = sp.tile([128, 1], F32, tag="c")  # final row scale r * s_true
                nc.vector.tensor_tensor(c_t, st[j]["r"], s_tr, mybir.AluOpType.mult)
                qa_t = qap.tile([128, I], BF16, tag="qa")
                tr = tmpf.tile([128, I], F32, tag="tmpr")
                # chunk only the last row (shortens its un-hideable serial
                # chain; chunking every row costs more in per-op overhead)
                blocks = ((0, 512), (512, 512), (1024, 384)) if chunked \
                    else ((0, I),)
                for boff, bw in blocks:
                    nc.vector.tensor_scalar(
                        tr[:, ds(boff, bw)], act_t[:, ds(boff, bw)], sc_eff, TWO23,
                        mybir.AluOpType.mult, mybir.AluOpType.add,
                    )
                    nc.scalar.activation(
                        qa_t[:, ds(boff, bw)], tr[:, ds(boff, bw)],
                        mybir.ActivationFunctionType.Identity, bias=neg223,
                    )
                st[j]["c"] = c_t
                st[j]["qa"] = qa_t

            def qatrans(j):
                """Transpose requantized act to contraction-major for fc2.

                PE for the fill row (scalar ring still busy) and the last row
                (latency-critical epilogue, tensor idle); DMA xbar otherwise."""
                qa_t = st[j]["qa"]
                qa_kt = qatp.tile([128, KT2, 128], BF16, tag="qakt")
                if 1 <= j < nt - 1:
                    nc.scalar.dma_start_transpose(qa_kt, qa_t)
                else:
                    for b in range(3):
                        nb = 4 if b < 2 else 3
                        pt = pst.tile([128, 4, 128], BF16, tag="pt")
                        for jj in range(nb):
                            nc.tensor.transpose(
                                pt[:, jj, :], qa_t[:, ts(4 * b + jj, 128)],
                                gl["ident"],
                            )
                        nc.scalar.copy(
                            qa_kt[:, ds(4 * b, nb), :], pt[:, ds(0, nb), :]
                        )
                st[j]["qakt"] = qa_kt

            def fc2_row(j, fine_store=False):
                """fc2 int8 GEMM (s2w folded into weights) + gate scale + store."""
                t0 = j * 128
                qa_kt = st[j]["qakt"]
                c_t = st[j]["c"]
                for c in range(4):
                    pa = ps2.tile([128, 512], F32, tag="pa")
                    for k in range(KT2):
                        w = w2_sb[(c, 0 if k < 6 else 1)]
                        nc.tensor.matmul(
                            pa, qa_kt[:, k, :], w[:, k if k < 6 else k - 6, :],
                            start=(k == 0), stop=(k == KT2 - 1),
                        )
                    yc = yp.tile([128, 512], F32, tag="y")
                    # finer mul+store chunks for the last row so the final
                    # DMA (nothing left to overlap it) is small
                    nch = 2 if fine_store else 1
                    for s in range(nch):
                        sl = ds(512 * c + 256 * s, 512 // nch)
                        yl = ds(256 * s, 512 // nch)
                        nc.vector.tensor_scalar_mul(yc[:, yl], pa[:, yl], c_t)
                        nc.scalar.dma_start(y_d[:][ds(t0, 128), sl], yc[:, yl])

            # Software-pipelined issue order: row i's fc1 runs before row
            # i-1's fc2 on the tensor queue, so row i-1's requant chain
            # (vector+scalar) hides under row i's fc1 matmuls.
            load(0)
            load(1)
            load_small()
            load_weights()
            quant(0)
            xtrans(0)
            for i in range(nt):
                load(i + 2)
                quant(i + 1)
                xtrans(i + 1)
                if i >= 1:
                    requant(i - 1)
                fc1_group(i, 0)
                fc1_group(i, 1)
                if i >= 1:
                    qatrans(i - 1)
                fc1_group(i, 2)
                if i >= 1:
                    fc2_row(i - 1)
            requant(nt - 1, chunked=True)
            qatrans(nt - 1)
            fc2_row(nt - 1, fine_store=True)

    nc.finalize()
    return nc


def kernel(hidden_states, gate_weight, w13_weight, w13_weight_scale,
           w2_weight, w2_weight_scale):
    x = np.ascontiguousarray(np.asarray(hidden_states, dtype=np.float32))
    gw = np.asarray(gate_weight, dtype=np.float32)
    w13 = np.asarray(w13_weight)
    s13 = np.ascontiguousarray(np.asarray(w13_weight_scale, dtype=np.float32))
    w2 = np.asarray(w2_weight)
    s2w = np.ascontiguousarray(np.asarray(w2_weight_scale, dtype=np.float32))

    # ---- host routing: fp32 gate, softmax, top-2, renormalize ----
    logits = (x @ gw.T).astype(np.float32)
    p = np.exp(logits - logits.max(axis=1, keepdims=True), dtype=np.float32)
    p = (p / p.sum(axis=1, keepdims=True)).astype(np.float32)
    topi = np.argsort(-p, axis=1, kind="stable")[:, :TOPK]  # ties -> lower index
    topv = np.take_along_axis(p, topi, axis=1).astype(np.float32)
    gates = (topv / topv.sum(axis=1, keepdims=True)).astype(np.float32)

    idxs, rvals = [], []
    for e in range(E):
        sel = topi == e
        tok = np.nonzero(sel.any(axis=1))[0]
        r = (gates * sel)[tok].sum(axis=1).astype(np.float32)
        idxs.append(tok)
        rvals.append(r)

    cap = max(128, max(len(t) for t in idxs))
    C = ((cap + 127) // 128) * 128

    if C not in _cache:
        _cache[C] = _build_program(C)
    nc = _cache[C]

    in_maps = []
    for e in range(E):
        n_e = len(idxs[e])
        xg = np.zeros((C, H), dtype=np.float32)
        xg[:n_e] = x[idxs[e]]
        rg = np.zeros((C, 1), dtype=np.float32)
        rg[:n_e, 0] = rvals[e]
        # fc1 weights, contraction-major, columns permuted group-wise
        # [g0|u0|g1|u1|g2|u2] so each group's DMA chunk is contiguous.
        w13t = np.ascontiguousarray(w13[e].T)        # [H, 2I] int8
        w13k = w13t.reshape(KT1, 128, 2 * I)
        im = {
            "x": xg, "r": rg,
            "s13g": np.ascontiguousarray(
                np.broadcast_to(s13[e][:I], (128, I)).astype(np.float32)),
            "s13u": np.ascontiguousarray(
                np.broadcast_to(s13[e][I:], (128, I)).astype(np.float32)),
            "ident": np.eye(128, dtype=ml_dtypes.bfloat16),
        }
        for g, (goff, cw) in enumerate(FC1_GROUPS):
            blk = np.concatenate(
                [w13k[:, :, goff : goff + cw], w13k[:, :, I + goff : I + goff + cw]],
                axis=2,
            )
            im[f"w13{g}"] = np.ascontiguousarray(blk).astype(ml_dtypes.bfloat16)
        # fc2 weights with s2w folded in (bf16), contraction-major, one
        # contiguous array per 512-column chunk
        w2f = w2[e].astype(np.float32) * s2w[e][:, None]   # [H, I]
        w2t = np.ascontiguousarray(w2f.T).reshape(KT2, 128, H)
        for c in range(4):
            im[f"w2c{c}"] = np.ascontiguousarray(
                w2t[:, :, 512 * c : 512 * (c + 1)]
            ).astype(ml_dtypes.bfloat16)
        in_maps.append(im)

    trace = bool(int(os.environ.get("MOE_TRACE", "0")))
    br = run_bass_kernel_spmd(nc, in_maps, list(range(E)), trace=trace)
    global LAST_EXEC_NS
    LAST_EXEC_NS = br.exec_time_ns
    res = br.results

    out = np.zeros((T, H), dtype=np.float32)
    for e in range(E):
        n_e = len(idxs[e])
        if n_e:
            out[idxs[e]] += np.asarray(res[e]["y"])[:n_e]
    return out
